# revision 42
# baseline (speedup 1.0000x reference)
"""DINOBevAligner Trainium2 kernel (8 NeuronCores, SPMD).

Host (numpy, derived ONLY from small inputs: lidar2img/w_view/logits/ln_*):
  * BEVFormer projection -> per-point validity, bilinear corner rows+weights.
  * Full algebraic pushdown of LayerNorm, view reduction and group reducer:
      y(q,cc) = [ sum_p lam'_p rstd_p Xg(p,cc) - T(q) Gam(cc) + U(q) Bet(cc) ]
    with lam' = softplus(w_view)*valid/(count*den) host-known, rstd device-
    computed, Xg the gamma*softmax(logits)-group-folded sampled features.
  * Q sharded over 8 cores (800 q each, 7 q-windows of 128). The union of
    feature rows referenced by one core is tiny (<=128 of 8214!), so all
    windows share ONE gathered 128-row chunk (superwindow packing).
  * Host metadata: one-hot weight matrices W1/W1^T (rows x points) and R0
    (points x q-slots). All last_tokens-touching math stays on device.

Device (per core, bf16 compute):
  * dma_gather 128 rows of last_tokens (normal + transposed layout).
  * Prologue: F~ = fold3(F*gamma_wg)+rowmean column; Gram G = F F^T/768 on
    PE; GW1 = G @ W1 batched.
  * Per 128-point tile (3 tiny PE matmuls + 2 DVE ops):
      mu_col = W1^T rm ; ip = W1 .* GW1 ; qf_col = ip^T 1  (E[x^2] via Gram)
  * Batched stats: rstd = 1/sqrt(qf - mu^2 + eps) in column space.
  * Associativity collapse: win = sum_t R_t^T (W1_t^T F~) = M^T F~ with
    M = sum_t W1_t @ (R0_t * rstd) accumulated in PSUM -> ONE [rows x q]
    matmul chain per window, no per-tile feature materialization at all.
  * Finalize: y = win - T*GamRep + U*BetRep; PE-transpose to [cc, q];
    DMA -> out [256, 896].
Host: concat per-core outputs -> (1, 256, 80, 80).

Measured: ~65 us HW exec on 8 NeuronCores, rel_err 3.9e-3.
"""
import numpy as np
import ml_dtypes

# ---------------- constants (hardcoded per spec) ----------------
B, V, Hp, Wp, C = 1, 6, 37, 37, 768
BEV_H, BEV_W, D = 80, 80, 4
Q = BEV_H * BEV_W
C_CTX, G = 256, 3
PC = (-51.2, -51.2, -5.0, 51.2, 51.2, 3.0)
Z = 8
SCALE, PAD_T, PAD_L = 0.32375, 113.0, 0.0
H2, W2 = 518.0, 518.0
EPS = 1e-6
LN_EPS = 1e-5
N_CORES = 8
QC = Q // N_CORES          # 800 queries per core
WIN = 128                  # q-window size (PSUM partition dim)
NW = (QC + WIN - 1) // WIN # 7 windows per core
NROW = V * Hp * Wp         # 8214 global feature rows

_BF16 = ml_dtypes.bfloat16


def _softplus(x):
    return np.log1p(np.exp(-np.abs(x))) + np.maximum(x, 0)


def _host_metadata(lidar2img, w_view, logits, ln_gamma, ln_beta):
    """Projection & scheduling metadata. Touches only small inputs."""
    f32 = np.float32
    zs = (np.linspace(0.5, Z - 0.5, D, dtype=f32) / Z)
    xs = (np.linspace(0.5, BEV_W - 0.5, BEV_W, dtype=f32) / BEV_W)
    ys = (np.linspace(0.5, BEV_H - 0.5, BEV_H, dtype=f32) / BEV_H)
    g = np.stack(np.broadcast_arrays(xs[None, None, :], ys[None, :, None],
                                     zs[:, None, None]), -1)
    ref = g.reshape(D, Q, 3).astype(f32)
    lo = np.array(PC[:3], f32)
    hi = np.array(PC[3:], f32)
    pts = ref * (hi - lo) + lo
    pts_h = np.concatenate([pts, np.ones_like(pts[..., :1])], -1)      # (D,Q,4)
    cam = np.einsum('vij,dqj->vdqi', lidar2img[0].astype(f32), pts_h)  # (V,D,Q,4)
    depth = cam[..., 2]
    bev_mask = depth > 1e-5
    uv = cam[..., 0:2] / np.maximum(depth, 1e-5)[..., None]
    u_d = uv[..., 0] * SCALE + PAD_L
    v_d = uv[..., 1] * SCALE + PAD_T
    valid = bev_mask & (u_d >= 0) & (u_d <= W2 - 1) & (v_d >= 0) & (v_d <= H2 - 1)
    x = (u_d / f32(W2 - 1.0) * (Wp - 1)).astype(f32)
    y = (v_d / f32(H2 - 1.0) * (Hp - 1)).astype(f32)

    wv = _softplus(w_view[0, :, 0].astype(np.float64)).astype(f32)      # (V,)
    den = max(float(wv.sum()), EPS)
    dn = np.maximum(valid.sum(1), EPS).astype(f32)                      # (V,Q)
    lam = (wv[:, None, None] / (dn[:, None, :] * den)) * valid          # (V,D,Q)

    lg = logits.astype(f32)
    wg = np.exp(lg - lg.max(-1, keepdims=True))
    wg = (wg / wg.sum(-1, keepdims=True)).astype(f32)                   # (256,3)
    Gam = (wg * ln_gamma.reshape(C_CTX, G)).sum(-1).astype(f32)
    Bet = (wg * ln_beta.reshape(C_CTX, G)).sum(-1).astype(f32)
    gw = (wg.reshape(-1) * ln_gamma.astype(f32)).astype(f32)            # (768,)
    Uq = lam.sum((0, 1)).astype(f32)                                    # (Q,)

    # corner rows/weights for valid points, vectorized
    x0 = np.floor(x).astype(np.int64)
    y0 = np.floor(y).astype(np.int64)
    wx1 = (x - x0).astype(f32)
    wy1 = (y - y0).astype(f32)
    corner_rows = np.zeros((4,) + x0.shape, np.int64)
    corner_wts = np.zeros((4,) + x0.shape, f32)
    for k, ((dx, dy), wt) in enumerate([
            ((0, 0), (1 - wx1) * (1 - wy1)), ((1, 0), wx1 * (1 - wy1)),
            ((0, 1), (1 - wx1) * wy1), ((1, 1), wx1 * wy1)]):
        xi = x0 + dx
        yi = y0 + dy
        inb = (xi >= 0) & (xi <= Wp - 1) & (yi >= 0) & (yi <= Hp - 1)
        r = np.clip(yi, 0, Hp - 1) * Wp + np.clip(xi, 0, Wp - 1)
        corner_rows[k] = (np.arange(V)[:, None, None] * (Hp * Wp) + r)
        corner_wts[k] = wt * inb
    # zero weights for invalid points so they are never emitted
    corner_wts *= valid[None].astype(f32)

    return dict(valid=valid, lam=lam, den=den, Gam=Gam, Bet=Bet, gw=gw, Uq=Uq,
                corner_rows=corner_rows, corner_wts=corner_wts)


def _build_schedule(meta):
    """Per-core per-window point lists + aligned structure.

    Returns structure (shared): ntw[w] tiles per window, nch[w] chunks per
    window; and per-core data arrays.
    """
    valid = meta['valid']
    lam = meta['lam']
    crows = meta['corner_rows']
    cwts = meta['corner_wts']

    # per (core, window): points (v, d, qlocal-in-window)
    core_win_pts = [[[] for _ in range(NW)] for _ in range(N_CORES)]
    vv, dd, qq = np.nonzero(valid)
    order = np.lexsort((dd, qq, vv))  # sort by view, then q, then d
    for i in order:
        v, d, q = int(vv[i]), int(dd[i]), int(qq[i])
        c = q // QC
        ql = q - c * QC
        w = ql // WIN
        core_win_pts[c][w].append((v, d, q, ql - w * WIN))

    # per (core, window) compact rows
    core_win_rows = [[None] * NW for _ in range(N_CORES)]
    for c in range(N_CORES):
        for w in range(NW):
            rows = set()
            for (v, d, q, _) in core_win_pts[c][w]:
                for k in range(4):
                    if cwts[k, v, d, q] != 0.0:
                        rows.add(int(crows[k, v, d, q]))
            core_win_rows[c][w] = sorted(rows)

    # shared structure
    ntw = [max(max((len(core_win_pts[c][w]) + 127) // 128, 1)
               for c in range(N_CORES)) for w in range(NW)]
    maxrows = max(len(core_win_rows[c][w])
                  for c in range(N_CORES) for w in range(NW))
    assert maxrows <= 128, f"window row set {maxrows} > 128 unsupported"
    # pack consecutive windows into superwindows sharing one 128-row chunk
    # (valid only if EVERY core's row union stays <= 128)
    packs = []
    cur = [0]
    for w in range(1, NW):
        ok = all(len(set().union(*[set(core_win_rows[c][x]) for x in cur + [w]]))
                 <= 128 for c in range(N_CORES))
        if ok:
            cur.append(w)
        else:
            packs.append(cur)
            cur = [w]
    packs.append(cur)
    return core_win_pts, core_win_rows, ntw, packs


def _build_core_arrays(core, meta, core_win_pts, core_win_rows, ntw, packs):
    """Build the per-core DRAM input arrays in execution order.

    packs: superwindow packing — each pack's windows share one 128-row chunk.
    """
    rp = 128
    lam = meta['lam']
    crows = meta['corner_rows']
    cwts = meta['corner_wts']
    NT = sum(ntw)
    NSW = len(packs)

    nidx = NSW * rp
    fidx = np.zeros((128, nidx // 16), np.int16)
    w1 = np.zeros((rp, NT * 128), _BF16)
    r0 = np.zeros((128, NT * 128), _BF16)
    ub = np.zeros((128, NW * C_CTX), np.float32)
    q_lo = core * QC

    w1t = np.zeros((128, NT * 128), _BF16)
    rows_all = np.zeros(nidx, np.int64)
    ti = 0
    for si, pack in enumerate(packs):
        rows = sorted(set().union(*[set(core_win_rows[core][x]) for x in pack]))
        assert len(rows) <= rp
        slot = {r: j for j, r in enumerate(rows)}
        rows_all[si * rp:si * rp + len(rows)] = rows
        for w in pack:
            pts = core_win_pts[core][w]
            for t in range(ntw[w]):
                tp = pts[t * 128:(t + 1) * 128]
                for p_i, (v, d, q, qsl) in enumerate(tp):
                    r0[p_i, ti * 128 + qsl] = lam[v, d, q]
                    for k in range(4):
                        wgt = cwts[k, v, d, q]
                        if wgt == 0.0:
                            continue
                        kk = slot[int(crows[k, v, d, q])]
                        w1[kk, ti * 128 + p_i] = np.float32(w1[kk, ti * 128 + p_i]) + wgt
                        w1t[p_i, ti * 128 + kk] = w1[kk, ti * 128 + p_i]
                ti += 1
            qs = q_lo + w * WIN
            qe = min(qs + WIN, q_lo + QC)
            ub[:qe - qs, w * C_CTX:(w + 1) * C_CTX] = (
                meta['Uq'][qs:qe, None] * meta['Bet'][None, :])
    for j in range(nidx):
        fidx[(j % 16)::16, j // 16] = rows_all[j]
    return dict(fidx=fidx, w1=w1, w1t=w1t, r0=r0, ub=ub)


def _build_bass(ntw, packs):
    """Build the SPMD bass kernel (structure shared across cores).

    V3: superwindows — consecutive q-windows share one 128-row feature
    chunk (row sets are tiny), so gathers / G / F~ / GW1 run once per
    superwindow. Gram-matrix variance, rowmean as matmul column; all
    gathers and prologues are emitted before per-window compute so the
    in-order engines never block the pipeline.
    """
    import concourse.bacc as bacc
    import concourse.tile as tile
    from concourse import mybir

    rp = 128
    NT = sum(ntw)
    NSW = len(packs)
    nts = [sum(ntw[w] for w in pack) for pack in packs]   # tiles per superwindow
    NTS_MAX = max(nts)
    NTW_MAX = max(ntw)
    NIDX = NSW * rp
    NCC = C // 128
    bf16 = mybir.dt.bfloat16
    f32 = mybir.dt.float32
    SQ = mybir.ActivationFunctionType.Sqrt
    CP = mybir.ActivationFunctionType.Copy

    nc = bacc.Bacc("TRN2", debug=False, num_devices=N_CORES)
    lt = nc.dram_tensor("ltb", [NROW, C], bf16, kind="ExternalInput")
    fidx_d = nc.dram_tensor("fidx", [128, NIDX // 16], mybir.dt.int16, kind="ExternalInput")
    w1_d = nc.dram_tensor("w1", [rp, NT * 128], bf16, kind="ExternalInput")
    w1t_d = nc.dram_tensor("w1t", [128, NT * 128], bf16, kind="ExternalInput")
    r0_d = nc.dram_tensor("r0", [128, NT * 128], bf16, kind="ExternalInput")
    ub_d = nc.dram_tensor("ub", [128, NW * C_CTX], f32, kind="ExternalInput")
    gam_d = nc.dram_tensor("gamrep", [128, C_CTX], f32, kind="ExternalInput")
    gw_d = nc.dram_tensor("gwrep", [128, C], f32, kind="ExternalInput")
    id_d = nc.dram_tensor("ident", [128, 128], f32, kind="ExternalInput")
    out_d = nc.dram_tensor("out", [C_CTX, NW * WIN], f32, kind="ExternalOutput")

    with tile.TileContext(nc) as tc:
        with (
            tc.tile_pool(name="const", bufs=1) as constp,
            tc.tile_pool(name="fstage", bufs=1) as fstage,
            tc.tile_pool(name="fext", bufs=1) as fextp,
            tc.tile_pool(name="gsb", bufs=2) as gsbp,
            tc.tile_pool(name="stats", bufs=2) as statsp,
            tc.tile_pool(name="xmu", bufs=NT + 4) as xmup,
            tc.tile_pool(name="rsc", bufs=8) as rscp,
            tc.tile_pool(name="ysb", bufs=3) as ysbp,
            tc.tile_pool(name="psx", bufs=2, space="PSUM") as psxp,
            tc.tile_pool(name="pswin", bufs=3, space="PSUM") as pswinp,
            tc.tile_pool(name="pssm", bufs=2, space="PSUM") as pssmp,
            tc.tile_pool(name="psqf", bufs=1, space="PSUM") as psqfp,
        ):
            # kick the GPSIMD ucode library load immediately (it takes
            # ~13us and otherwise blocks the first dma_gather)
            from concourse import library_config
            nc.gpsimd.load_library(library_config.mlp)
            # fidx first, then gathers — everything else queues behind
            fidx_sb = constp.tile([128, NIDX // 16], mybir.dt.int16)
            nc.sync.dma_start(fidx_sb[:], fidx_d[:])

            # --- all gathers first (Pool is in-order) ---
            fws, ftws = [], []
            for si in range(NSW):
                fw = fstage.tile([128, 1, C], bf16, tag=f"fw{si}")
                nc.gpsimd.dma_gather(fw[:], lt[:], fidx_sb[:, si * 8:(si + 1) * 8],
                                     num_idxs=rp, num_idxs_reg=rp, elem_size=C)
                fws.append(fw)
                ftw = fstage.tile([128, NCC, rp], bf16, tag=f"ftw{si}")
                nc.gpsimd.dma_gather(ftw[:], lt[:], fidx_sb[:, si * 8:(si + 1) * 8],
                                     num_idxs=rp, num_idxs_reg=rp, elem_size=C,
                                     transpose=True)
                ftws.append(ftw)

            w1_sb = constp.tile([rp, NT * 128], bf16)
            nc.sync.dma_start(w1_sb[:, 0:8 * 128], w1_d[:, 0:8 * 128])
            nc.sync.dma_start(w1_sb[:, 8 * 128:], w1_d[:, 8 * 128:])
            w1t_sb = constp.tile([128, NT * 128], bf16)
            nc.sync.dma_start(w1t_sb[:], w1t_d[:])
            r0_sb = constp.tile([128, NT * 128], bf16)
            nc.sync.dma_start(r0_sb[:], r0_d[:])
            ub_sb = constp.tile([128, NW * C_CTX], f32)
            nc.scalar.dma_start(ub_sb[:], ub_d[:])
            gam_sb = constp.tile([128, C_CTX], f32)
            nc.scalar.dma_start(gam_sb[:], gam_d[:])
            gw_sb = constp.tile([128, C], f32)
            nc.scalar.dma_start(gw_sb[:], gw_d[:])
            id_sb = constp.tile([128, 128], f32)
            nc.scalar.dma_start(id_sb[:], id_d[:])
            eps_sb = constp.tile([128, 1], f32)
            nc.vector.memset(eps_sb[:], LN_EPS)
            ones_sb = constp.tile([128, 1], bf16)
            nc.vector.memset(ones_sb[:], 1.0)

            # --- per-superwindow prologues (before any tile work) ---
            fes, gsbs, gw1s = [], [], []
            ti0 = [0] * NSW
            t_acc = 0
            for si, pack in enumerate(packs):
                ti0[si] = t_acc
                t_acc += nts[si]
            for si, pack in enumerate(packs):
                F = fws[si][:, 0, :]
                # F~ = fold3(F * gamma*wg) -> fe[:, 0:256] bf16; rm col 256
                fgg = fstage.tile([128, C], f32, tag=f"fgg{si}")
                nc.vector.tensor_mul(fgg[:], F, gw_sb[:])
                f3 = fgg[:].rearrange("p (a g) -> p a g", g=3)
                ft = fstage.tile([128, C_CTX], f32, tag=f"ftm{si}")
                nc.vector.tensor_add(ft[:], f3[:, :, 0], f3[:, :, 1])
                fe = fextp.tile([128, C_CTX + 1], bf16, tag=f"fext{si}")
                nc.vector.tensor_add(fe[:, 0:C_CTX], ft[:], f3[:, :, 2])
                rmf = fstage.tile([128, 1], f32, tag=f"rmf{si}")
                nc.vector.reduce_sum(rmf[:], F, axis=mybir.AxisListType.X)
                nc.scalar.activation(fe[:, C_CTX:C_CTX + 1], rmf[:], CP,
                                     scale=1.0 / C)
                fes.append(fe)
                # G = F F^T / 768
                gps = pssmp.tile([128, 128], f32, tag="sm")
                for ci in range(NCC):
                    fts = ftws[si][:, ci, :]
                    nc.tensor.matmul(gps[:, 0:128], fts, fts,
                                     start=(ci == 0), stop=(ci == NCC - 1))
                gsb = gsbp.tile([128, 128], bf16, tag=f"gsb{si}")
                nc.scalar.activation(gsb[:], gps[:, 0:128], CP, scale=1.0 / C)
                gsbs.append(gsb)
                # batched GW1 over the whole superwindow
                nw128 = nts[si] * 128
                gw1 = gsbp.tile([128, nw128], bf16, tag=f"gw1{si}")
                bounds = [0, 128] + list(range(512, nw128, 512)) + [nw128]
                for k, (h, he) in enumerate(zip(bounds[:-1], bounds[1:])):
                    ps = pssmp.tile([128, 512], f32, tag="sm")
                    nc.tensor.matmul(ps[:, 0:he - h], gsb[:],
                                     w1_sb[0:128, ti0[si] * 128 + h:ti0[si] * 128 + he],
                                     start=True, stop=True)
                    if k % 2 == 0:
                        nc.scalar.copy(gw1[:, h:he], ps[:, 0:he - h])
                    else:
                        nc.vector.tensor_copy(gw1[:, h:he], ps[:, 0:he - h])
                gw1s.append(gw1)

            # --- per-superwindow compute ---
            # Associativity: win = sum_t R_t^T (W1_t^T Fext) = M^T Fext with
            # M = sum_t W1_t @ R_t accumulated in PSUM [rows, q]. Per tile
            # only 3 tiny matmuls (mu, qf, M); one win matmul per window.
            ti = 0
            for si, pack in enumerate(packs):
                fe = fes[si]
                gw1 = gw1s[si]
                ipall = gsbp.tile([128, nts[si] * 128], bf16, tag=f"ip{si}")
                nt_s = nts[si]
                qfmu = psqfp.tile([128, 2 * NTS_MAX], f32, tag="qfmu")
                for st in range(nt_s):
                    sl = slice((ti + st) * 128, (ti + st + 1) * 128)
                    tsl = slice(st * 128, (st + 1) * 128)
                    # mu column straight into qfmu
                    nc.tensor.matmul(qfmu[:, NTS_MAX + st:NTS_MAX + st + 1],
                                     w1_sb[0:128, sl], fe[:, C_CTX:C_CTX + 1],
                                     start=True, stop=True)
                    # ip = W1 .* GW1 ; qf column
                    nc.vector.tensor_mul(ipall[:, tsl], w1_sb[0:128, sl],
                                         gw1[:, tsl])
                    nc.tensor.matmul(qfmu[:, st:st + 1], ipall[:, tsl],
                                     ones_sb[:], start=True, stop=True)

                # superwindow stats: rstd = 1/sqrt(qf - mu^2 + eps)
                musq = statsp.tile([128, NTS_MAX], f32, tag="musq")
                nc.scalar.square(musq[:, 0:nt_s], qfmu[:, NTS_MAX:NTS_MAX + nt_s])
                rstdT = statsp.tile([128, NTS_MAX], f32, tag="rstdT")
                nc.vector.tensor_sub(rstdT[:, 0:nt_s], qfmu[:, 0:nt_s],
                                     musq[:, 0:nt_s])
                nc.scalar.activation(rstdT[:, 0:nt_s], rstdT[:, 0:nt_s], SQ,
                                     bias=eps_sb[:])
                nc.vector.reciprocal(rstdT[:, 0:nt_s], rstdT[:, 0:nt_s])

                # per window: accumulate M, then win = M^T @ Fext
                st = 0
                for w in pack:
                    mps = psxp.tile([128, 128], f32, tag="psx")
                    for t in range(ntw[w]):
                        sl = slice((ti + st) * 128, (ti + st + 1) * 128)
                        rsc = rscp.tile([128, 128], bf16, tag="rsc")
                        nc.vector.tensor_mul(rsc[:], r0_sb[:, sl],
                                             rstdT[:, st:st + 1].broadcast_to((128, 128)))
                        nc.tensor.matmul(mps[:], w1t_sb[:, sl], rsc[:],
                                         start=(t == 0), stop=(t == ntw[w] - 1))
                        st += 1
                    msb = rscp.tile([128, 128], bf16, tag="msb")
                    nc.scalar.copy(msb[:], mps[:])
                    win_ps = pswinp.tile([128, C_CTX + 1], f32, tag="win")
                    nc.tensor.matmul(win_ps[:], msb[:], fe[:],
                                     start=True, stop=True)
                    tsb = statsp.tile([128, 1], f32, tag="tsb")
                    nc.scalar.copy(tsb[:], win_ps[:, C_CTX:C_CTX + 1])
                    gt = ysbp.tile([128, C_CTX], f32, tag="gt")
                    nc.scalar.activation(gt[:], gam_sb[:], CP, scale=tsb[:])
                    ysb = ysbp.tile([128, C_CTX], f32, tag="ysb")
                    nc.vector.tensor_sub(ysb[:], win_ps[:, 0:C_CTX], gt[:])
                    nc.vector.tensor_add(ysb[:], ysb[:],
                                         ub_sb[:, w * C_CTX:(w + 1) * C_CTX])
                    for h in range(2):
                        ptr = pssmp.tile([128, 128], f32, tag="sm")
                        nc.tensor.transpose(ptr[:], ysb[:, h * 128:(h + 1) * 128],
                                            id_sb[:])
                        yt = ysbp.tile([128, 128], f32, tag="yt")
                        nc.scalar.copy(yt[:], ptr[:])
                        nc.sync.dma_start(
                            out_d[h * 128:(h + 1) * 128, w * WIN:(w + 1) * WIN],
                            yt[:])
                ti += nts[si]
    nc.compile()
    return nc


TRACE = False          # set by test.py to capture an NTFF profile
TRACE_KW = {}
LAST_RESULT = None     # BassKernelResults stash for test.py


def kernel(**inputs):
    from concourse.bass_utils import run_bass_kernel_spmd

    last_tokens = np.ascontiguousarray(inputs["last_tokens"], dtype=np.float32)
    meta = _host_metadata(inputs["lidar2img"], inputs["w_view"], inputs["logits"],
                          inputs["ln_gamma"], inputs["ln_beta"])
    core_win_pts, core_win_rows, ntw, packs = _build_schedule(meta)
    nc = _build_bass(ntw, packs)

    gamrep = np.broadcast_to(meta['Gam'][None, :], (128, C_CTX)).copy()
    gwrep = np.broadcast_to(meta['gw'][None, :], (128, C)).copy()
    ident = np.eye(128, dtype=np.float32)
    ltb = last_tokens.reshape(NROW, C).astype(_BF16)

    in_maps = []
    for c in range(N_CORES):
        arr = _build_core_arrays(c, meta, core_win_pts, core_win_rows, ntw, packs)
        in_maps.append({
            "ltb": ltb,
            "fidx": arr['fidx'],
            "w1": arr['w1'],
            "w1t": arr['w1t'],
            "r0": arr['r0'],
            "ub": arr['ub'],
            "gamrep": gamrep,
            "gwrep": gwrep,
            "ident": ident,
        })

    res = run_bass_kernel_spmd(nc, in_maps, core_ids=list(range(N_CORES)),
                               trace=TRACE, **TRACE_KW)
    global LAST_RESULT
    LAST_RESULT = res
    out = np.zeros((Q, C_CTX), np.float32)
    for c in range(N_CORES):
        y = res.results[c]["out"]          # [256, 896]
        out[c * QC:(c + 1) * QC] = y[:, :QC].T
    return out.T.reshape(1, C_CTX, BEV_H, BEV_W).copy()


if __name__ == "__main__":
    ins = dict(np.load('/root/problem/inputs_cache.npz'))
    expected = np.load('/root/problem/expected_cache.npy')
    got = kernel(**ins)
    err = np.linalg.norm(got - expected) / np.linalg.norm(expected)
    print("rel_err:", err)


# revision 43
# speedup vs baseline: 1.0616x; 1.0616x over previous
"""DINOBevAligner Trainium2 kernel (8 NeuronCores, SPMD).

Host (numpy, derived ONLY from small inputs: lidar2img/w_view/logits/ln_*):
  * BEVFormer projection -> per-point validity, bilinear corner rows+weights.
  * Full algebraic pushdown of LayerNorm, view reduction and group reducer:
      y(q,cc) = [ sum_p lam'_p rstd_p Xg(p,cc) - T(q) Gam(cc) + U(q) Bet(cc) ]
    with lam' = softplus(w_view)*valid/(count*den) host-known, rstd device-
    computed, Xg the gamma*softmax(logits)-group-folded sampled features.
  * Q sharded over 8 cores (800 q each, 7 q-windows of 128). The union of
    feature rows referenced by one core is tiny (<=128 of 8214!), so all
    windows share ONE gathered 128-row chunk (superwindow packing).
  * Host metadata: one-hot weight matrices W1/W1^T (rows x points) and R0
    (points x q-slots). All last_tokens-touching math stays on device.

Device (per core, bf16 compute):
  * dma_gather 128 rows of last_tokens (normal + transposed layout).
  * Prologue: F~ = fold3(F*gamma_wg)+rowmean column; Gram G = F F^T/768 on
    PE; GW1 = G @ W1 batched.
  * Per 128-point tile (3 tiny PE matmuls + 2 DVE ops):
      mu_col = W1^T rm ; ip = W1 .* GW1 ; qf_col = ip^T 1  (E[x^2] via Gram)
  * Batched stats: rstd = 1/sqrt(qf - mu^2 + eps) in column space.
  * Associativity collapse: win = sum_t R_t^T (W1_t^T F~) = M^T F~ with
    M = sum_t W1_t @ (R0_t * rstd) accumulated in PSUM -> ONE [rows x q]
    matmul chain per window, no per-tile feature materialization at all.
  * Finalize: y = win - T*GamRep + U*BetRep; PE-transpose to [cc, q];
    DMA -> out [256, 896].
Host: concat per-core outputs -> (1, 256, 80, 80).

Measured: ~65 us HW exec on 8 NeuronCores, rel_err 3.9e-3.
"""
import numpy as np
import ml_dtypes

# ---------------- constants (hardcoded per spec) ----------------
B, V, Hp, Wp, C = 1, 6, 37, 37, 768
BEV_H, BEV_W, D = 80, 80, 4
Q = BEV_H * BEV_W
C_CTX, G = 256, 3
PC = (-51.2, -51.2, -5.0, 51.2, 51.2, 3.0)
Z = 8
SCALE, PAD_T, PAD_L = 0.32375, 113.0, 0.0
H2, W2 = 518.0, 518.0
EPS = 1e-6
LN_EPS = 1e-5
N_CORES = 8
QC = Q // N_CORES          # 800 queries per core
WIN = 128                  # q-window size (PSUM partition dim)
NW = (QC + WIN - 1) // WIN # 7 windows per core
NROW = V * Hp * Wp         # 8214 global feature rows

_BF16 = ml_dtypes.bfloat16


def _softplus(x):
    return np.log1p(np.exp(-np.abs(x))) + np.maximum(x, 0)


def _host_metadata(lidar2img, w_view, logits, ln_gamma, ln_beta):
    """Projection & scheduling metadata. Touches only small inputs."""
    f32 = np.float32
    zs = (np.linspace(0.5, Z - 0.5, D, dtype=f32) / Z)
    xs = (np.linspace(0.5, BEV_W - 0.5, BEV_W, dtype=f32) / BEV_W)
    ys = (np.linspace(0.5, BEV_H - 0.5, BEV_H, dtype=f32) / BEV_H)
    g = np.stack(np.broadcast_arrays(xs[None, None, :], ys[None, :, None],
                                     zs[:, None, None]), -1)
    ref = g.reshape(D, Q, 3).astype(f32)
    lo = np.array(PC[:3], f32)
    hi = np.array(PC[3:], f32)
    pts = ref * (hi - lo) + lo
    pts_h = np.concatenate([pts, np.ones_like(pts[..., :1])], -1)      # (D,Q,4)
    cam = np.einsum('vij,dqj->vdqi', lidar2img[0].astype(f32), pts_h)  # (V,D,Q,4)
    depth = cam[..., 2]
    bev_mask = depth > 1e-5
    uv = cam[..., 0:2] / np.maximum(depth, 1e-5)[..., None]
    u_d = uv[..., 0] * SCALE + PAD_L
    v_d = uv[..., 1] * SCALE + PAD_T
    valid = bev_mask & (u_d >= 0) & (u_d <= W2 - 1) & (v_d >= 0) & (v_d <= H2 - 1)
    x = (u_d / f32(W2 - 1.0) * (Wp - 1)).astype(f32)
    y = (v_d / f32(H2 - 1.0) * (Hp - 1)).astype(f32)

    wv = _softplus(w_view[0, :, 0].astype(np.float64)).astype(f32)      # (V,)
    den = max(float(wv.sum()), EPS)
    dn = np.maximum(valid.sum(1), EPS).astype(f32)                      # (V,Q)
    lam = (wv[:, None, None] / (dn[:, None, :] * den)) * valid          # (V,D,Q)

    lg = logits.astype(f32)
    wg = np.exp(lg - lg.max(-1, keepdims=True))
    wg = (wg / wg.sum(-1, keepdims=True)).astype(f32)                   # (256,3)
    Gam = (wg * ln_gamma.reshape(C_CTX, G)).sum(-1).astype(f32)
    Bet = (wg * ln_beta.reshape(C_CTX, G)).sum(-1).astype(f32)
    gw = (wg.reshape(-1) * ln_gamma.astype(f32)).astype(f32)            # (768,)
    Uq = lam.sum((0, 1)).astype(f32)                                    # (Q,)

    # corner rows/weights for valid points, vectorized
    x0 = np.floor(x).astype(np.int64)
    y0 = np.floor(y).astype(np.int64)
    wx1 = (x - x0).astype(f32)
    wy1 = (y - y0).astype(f32)
    corner_rows = np.zeros((4,) + x0.shape, np.int64)
    corner_wts = np.zeros((4,) + x0.shape, f32)
    for k, ((dx, dy), wt) in enumerate([
            ((0, 0), (1 - wx1) * (1 - wy1)), ((1, 0), wx1 * (1 - wy1)),
            ((0, 1), (1 - wx1) * wy1), ((1, 1), wx1 * wy1)]):
        xi = x0 + dx
        yi = y0 + dy
        inb = (xi >= 0) & (xi <= Wp - 1) & (yi >= 0) & (yi <= Hp - 1)
        r = np.clip(yi, 0, Hp - 1) * Wp + np.clip(xi, 0, Wp - 1)
        corner_rows[k] = (np.arange(V)[:, None, None] * (Hp * Wp) + r)
        corner_wts[k] = wt * inb
    # zero weights for invalid points so they are never emitted
    corner_wts *= valid[None].astype(f32)

    return dict(valid=valid, lam=lam, den=den, Gam=Gam, Bet=Bet, gw=gw, Uq=Uq,
                corner_rows=corner_rows, corner_wts=corner_wts)


def _build_schedule(meta):
    """Per-core per-window point lists + aligned structure.

    Returns structure (shared): ntw[w] tiles per window, nch[w] chunks per
    window; and per-core data arrays.
    """
    valid = meta['valid']
    lam = meta['lam']
    crows = meta['corner_rows']
    cwts = meta['corner_wts']

    # per (core, window): points (v, d, qlocal-in-window)
    core_win_pts = [[[] for _ in range(NW)] for _ in range(N_CORES)]
    vv, dd, qq = np.nonzero(valid)
    order = np.lexsort((dd, qq, vv))  # sort by view, then q, then d
    for i in order:
        v, d, q = int(vv[i]), int(dd[i]), int(qq[i])
        c = q // QC
        ql = q - c * QC
        w = ql // WIN
        core_win_pts[c][w].append((v, d, q, ql - w * WIN))

    # per (core, window) compact rows
    core_win_rows = [[None] * NW for _ in range(N_CORES)]
    for c in range(N_CORES):
        for w in range(NW):
            rows = set()
            for (v, d, q, _) in core_win_pts[c][w]:
                for k in range(4):
                    if cwts[k, v, d, q] != 0.0:
                        rows.add(int(crows[k, v, d, q]))
            core_win_rows[c][w] = sorted(rows)

    # shared structure
    ntw = [max(max((len(core_win_pts[c][w]) + 127) // 128, 1)
               for c in range(N_CORES)) for w in range(NW)]
    maxrows = max(len(core_win_rows[c][w])
                  for c in range(N_CORES) for w in range(NW))
    assert maxrows <= 128, f"window row set {maxrows} > 128 unsupported"
    # pack consecutive windows into superwindows sharing one 128-row chunk
    # (valid only if EVERY core's row union stays <= 128)
    packs = []
    cur = [0]
    for w in range(1, NW):
        ok = all(len(set().union(*[set(core_win_rows[c][x]) for x in cur + [w]]))
                 <= 128 for c in range(N_CORES))
        if ok:
            cur.append(w)
        else:
            packs.append(cur)
            cur = [w]
    packs.append(cur)
    return core_win_pts, core_win_rows, ntw, packs


def _build_core_arrays(core, meta, core_win_pts, core_win_rows, ntw, packs):
    """Build the per-core DRAM input arrays in execution order.

    packs: superwindow packing — each pack's windows share one 128-row chunk.
    """
    rp = 128
    lam = meta['lam']
    crows = meta['corner_rows']
    cwts = meta['corner_wts']
    NT = sum(ntw)
    NSW = len(packs)

    nidx = NSW * rp
    fidx = np.zeros((128, nidx // 16), np.int16)
    w1 = np.zeros((rp, NT * 128), _BF16)
    r0 = np.zeros((128, NT * 128), _BF16)
    ub = np.zeros((128, NW * C_CTX), np.float32)
    q_lo = core * QC

    w1t = np.zeros((128, NT * 128), _BF16)
    rows_all = np.zeros(nidx, np.int64)
    ti = 0
    for si, pack in enumerate(packs):
        rows = sorted(set().union(*[set(core_win_rows[core][x]) for x in pack]))
        assert len(rows) <= rp
        slot = {r: j for j, r in enumerate(rows)}
        rows_all[si * rp:si * rp + len(rows)] = rows
        for w in pack:
            pts = core_win_pts[core][w]
            for t in range(ntw[w]):
                tp = pts[t * 128:(t + 1) * 128]
                for p_i, (v, d, q, qsl) in enumerate(tp):
                    r0[p_i, ti * 128 + qsl] = lam[v, d, q]
                    for k in range(4):
                        wgt = cwts[k, v, d, q]
                        if wgt == 0.0:
                            continue
                        kk = slot[int(crows[k, v, d, q])]
                        w1[kk, ti * 128 + p_i] = np.float32(w1[kk, ti * 128 + p_i]) + wgt
                        w1t[p_i, ti * 128 + kk] = w1[kk, ti * 128 + p_i]
                ti += 1
            qs = q_lo + w * WIN
            qe = min(qs + WIN, q_lo + QC)
            ub[:qe - qs, w * C_CTX:(w + 1) * C_CTX] = (
                meta['Uq'][qs:qe, None] * meta['Bet'][None, :])
    for j in range(nidx):
        fidx[(j % 16)::16, j // 16] = rows_all[j]
    return dict(fidx=fidx, w1=w1, w1t=w1t, r0=r0, ub=ub)


def _build_bass(ntw, packs):
    """Build the SPMD bass kernel (structure shared across cores).

    V3: superwindows — consecutive q-windows share one 128-row feature
    chunk (row sets are tiny), so gathers / G / F~ / GW1 run once per
    superwindow. Gram-matrix variance, rowmean as matmul column; all
    gathers and prologues are emitted before per-window compute so the
    in-order engines never block the pipeline.
    """
    import concourse.bacc as bacc
    import concourse.tile as tile
    from concourse import mybir

    rp = 128
    NT = sum(ntw)
    NSW = len(packs)
    nts = [sum(ntw[w] for w in pack) for pack in packs]   # tiles per superwindow
    NTS_MAX = max(nts)
    NTW_MAX = max(ntw)
    NIDX = NSW * rp
    NCC = C // 128
    bf16 = mybir.dt.bfloat16
    f32 = mybir.dt.float32
    SQ = mybir.ActivationFunctionType.Sqrt
    CP = mybir.ActivationFunctionType.Copy

    nc = bacc.Bacc("TRN2", debug=False, num_devices=N_CORES)
    lt = nc.dram_tensor("ltb", [NROW, C], bf16, kind="ExternalInput")
    fidx_d = nc.dram_tensor("fidx", [128, NIDX // 16], mybir.dt.int16, kind="ExternalInput")
    w1_d = nc.dram_tensor("w1", [rp, NT * 128], bf16, kind="ExternalInput")
    w1t_d = nc.dram_tensor("w1t", [128, NT * 128], bf16, kind="ExternalInput")
    r0_d = nc.dram_tensor("r0", [128, NT * 128], bf16, kind="ExternalInput")
    ub_d = nc.dram_tensor("ub", [128, NW * C_CTX], f32, kind="ExternalInput")
    gam_d = nc.dram_tensor("gamrep", [128, C_CTX], f32, kind="ExternalInput")
    gw_d = nc.dram_tensor("gwrep", [128, C], f32, kind="ExternalInput")
    id_d = nc.dram_tensor("ident", [128, 128], f32, kind="ExternalInput")
    out_d = nc.dram_tensor("out", [NW * WIN, C_CTX], f32, kind="ExternalOutput")

    with tile.TileContext(nc) as tc:
        with (
            tc.tile_pool(name="const", bufs=1) as constp,
            tc.tile_pool(name="fstage", bufs=1) as fstage,
            tc.tile_pool(name="fext", bufs=1) as fextp,
            tc.tile_pool(name="gsb", bufs=2) as gsbp,
            tc.tile_pool(name="stats", bufs=2) as statsp,
            tc.tile_pool(name="xmu", bufs=NT + 4) as xmup,
            tc.tile_pool(name="rsc", bufs=8) as rscp,
            tc.tile_pool(name="ysb", bufs=3) as ysbp,
            tc.tile_pool(name="psx", bufs=2, space="PSUM") as psxp,
            tc.tile_pool(name="pswin", bufs=3, space="PSUM") as pswinp,
            tc.tile_pool(name="pssm", bufs=2, space="PSUM") as pssmp,
            tc.tile_pool(name="psqf", bufs=1, space="PSUM") as psqfp,
        ):
            # kick the GPSIMD ucode library load immediately (it takes
            # ~13us and otherwise blocks the first dma_gather)
            from concourse import library_config
            nc.gpsimd.load_library(library_config.mlp)
            # fidx first, then gathers — everything else queues behind
            fidx_sb = constp.tile([128, NIDX // 16], mybir.dt.int16)
            nc.sync.dma_start(fidx_sb[:], fidx_d[:])

            # --- all gathers first (Pool is in-order) ---
            fws, ftws = [], []
            for si in range(NSW):
                fw = fstage.tile([128, 1, C], bf16, tag=f"fw{si}")
                nc.gpsimd.dma_gather(fw[:], lt[:], fidx_sb[:, si * 8:(si + 1) * 8],
                                     num_idxs=rp, num_idxs_reg=rp, elem_size=C)
                fws.append(fw)
                ftw = fstage.tile([128, NCC, rp], bf16, tag=f"ftw{si}")
                nc.gpsimd.dma_gather(ftw[:], lt[:], fidx_sb[:, si * 8:(si + 1) * 8],
                                     num_idxs=rp, num_idxs_reg=rp, elem_size=C,
                                     transpose=True)
                ftws.append(ftw)

            w1_sb = constp.tile([rp, NT * 128], bf16)
            nc.sync.dma_start(w1_sb[:, 0:8 * 128], w1_d[:, 0:8 * 128])
            nc.sync.dma_start(w1_sb[:, 8 * 128:], w1_d[:, 8 * 128:])
            w1t_sb = constp.tile([128, NT * 128], bf16)
            nc.sync.dma_start(w1t_sb[:], w1t_d[:])
            r0_sb = constp.tile([128, NT * 128], bf16)
            nc.sync.dma_start(r0_sb[:], r0_d[:])
            ub_sb = constp.tile([128, NW * C_CTX], f32)
            nc.scalar.dma_start(ub_sb[:], ub_d[:])
            gam_sb = constp.tile([128, C_CTX], f32)
            nc.scalar.dma_start(gam_sb[:], gam_d[:])
            gw_sb = constp.tile([128, C], f32)
            nc.scalar.dma_start(gw_sb[:], gw_d[:])
            id_sb = constp.tile([128, 128], f32)
            nc.scalar.dma_start(id_sb[:], id_d[:])
            eps_sb = constp.tile([128, 1], f32)
            nc.vector.memset(eps_sb[:], LN_EPS)
            ones_sb = constp.tile([128, 1], bf16)
            nc.vector.memset(ones_sb[:], 1.0)

            # --- per-superwindow prologues (before any tile work) ---
            fes, gsbs, gw1s = [], [], []
            ti0 = [0] * NSW
            t_acc = 0
            for si, pack in enumerate(packs):
                ti0[si] = t_acc
                t_acc += nts[si]
            for si, pack in enumerate(packs):
                F = fws[si][:, 0, :]
                # F~ = fold3(F * gamma*wg) -> fe[:, 0:256] bf16; rm col 256
                fgg = fstage.tile([128, C], f32, tag=f"fgg{si}")
                nc.vector.tensor_mul(fgg[:], F, gw_sb[:])
                f3 = fgg[:].rearrange("p (a g) -> p a g", g=3)
                ft = fstage.tile([128, C_CTX], f32, tag=f"ftm{si}")
                nc.vector.tensor_add(ft[:], f3[:, :, 0], f3[:, :, 1])
                fe = fextp.tile([128, C_CTX + 1], bf16, tag=f"fext{si}")
                nc.vector.tensor_add(fe[:, 0:C_CTX], ft[:], f3[:, :, 2])
                rmf = fstage.tile([128, 1], f32, tag=f"rmf{si}")
                nc.vector.reduce_sum(rmf[:], F, axis=mybir.AxisListType.X)
                nc.scalar.activation(fe[:, C_CTX:C_CTX + 1], rmf[:], CP,
                                     scale=1.0 / C)
                fes.append(fe)
                # G = F F^T / 768
                gps = pssmp.tile([128, 128], f32, tag="sm")
                for ci in range(NCC):
                    fts = ftws[si][:, ci, :]
                    nc.tensor.matmul(gps[:, 0:128], fts, fts,
                                     start=(ci == 0), stop=(ci == NCC - 1))
                gsb = gsbp.tile([128, 128], bf16, tag=f"gsb{si}")
                nc.scalar.activation(gsb[:], gps[:, 0:128], CP, scale=1.0 / C)
                gsbs.append(gsb)
                # batched GW1 over the whole superwindow
                nw128 = nts[si] * 128
                gw1 = gsbp.tile([128, nw128], bf16, tag=f"gw1{si}")
                bounds = [0, 128] + list(range(512, nw128, 512)) + [nw128]
                for k, (h, he) in enumerate(zip(bounds[:-1], bounds[1:])):
                    ps = pssmp.tile([128, 512], f32, tag="sm")
                    nc.tensor.matmul(ps[:, 0:he - h], gsb[:],
                                     w1_sb[0:128, ti0[si] * 128 + h:ti0[si] * 128 + he],
                                     start=True, stop=True)
                    if k % 2 == 0:
                        nc.scalar.copy(gw1[:, h:he], ps[:, 0:he - h])
                    else:
                        nc.vector.tensor_copy(gw1[:, h:he], ps[:, 0:he - h])
                gw1s.append(gw1)

            # --- per-superwindow compute ---
            # Associativity: win = sum_t R_t^T (W1_t^T Fext) = M^T Fext with
            # M = sum_t W1_t @ R_t accumulated in PSUM [rows, q]. Per tile
            # only 3 tiny matmuls (mu, qf, M); one win matmul per window.
            ti = 0
            for si, pack in enumerate(packs):
                fe = fes[si]
                gw1 = gw1s[si]
                ipall = gsbp.tile([128, nts[si] * 128], bf16, tag=f"ip{si}")
                nt_s = nts[si]
                qfmu = psqfp.tile([128, 2 * NTS_MAX], f32, tag="qfmu")
                for st in range(nt_s):
                    sl = slice((ti + st) * 128, (ti + st + 1) * 128)
                    tsl = slice(st * 128, (st + 1) * 128)
                    # mu column straight into qfmu
                    nc.tensor.matmul(qfmu[:, NTS_MAX + st:NTS_MAX + st + 1],
                                     w1_sb[0:128, sl], fe[:, C_CTX:C_CTX + 1],
                                     start=True, stop=True)
                    # ip = W1 .* GW1 ; qf column
                    nc.vector.tensor_mul(ipall[:, tsl], w1_sb[0:128, sl],
                                         gw1[:, tsl])
                    nc.tensor.matmul(qfmu[:, st:st + 1], ipall[:, tsl],
                                     ones_sb[:], start=True, stop=True)

                # superwindow stats: rstd = 1/sqrt(qf - mu^2 + eps)
                musq = statsp.tile([128, NTS_MAX], f32, tag="musq")
                nc.scalar.square(musq[:, 0:nt_s], qfmu[:, NTS_MAX:NTS_MAX + nt_s])
                rstdT = statsp.tile([128, NTS_MAX], f32, tag="rstdT")
                nc.vector.tensor_sub(rstdT[:, 0:nt_s], qfmu[:, 0:nt_s],
                                     musq[:, 0:nt_s])
                nc.scalar.activation(rstdT[:, 0:nt_s], rstdT[:, 0:nt_s], SQ,
                                     bias=eps_sb[:])
                nc.vector.reciprocal(rstdT[:, 0:nt_s], rstdT[:, 0:nt_s])

                # per window: accumulate M, then win = M^T @ Fext
                st = 0
                for w in pack:
                    mps = psxp.tile([128, 128], f32, tag="psx")
                    for t in range(ntw[w]):
                        sl = slice((ti + st) * 128, (ti + st + 1) * 128)
                        rsc = rscp.tile([128, 128], bf16, tag="rsc")
                        nc.vector.tensor_mul(rsc[:], r0_sb[:, sl],
                                             rstdT[:, st:st + 1].broadcast_to((128, 128)))
                        nc.tensor.matmul(mps[:], w1t_sb[:, sl], rsc[:],
                                         start=(t == 0), stop=(t == ntw[w] - 1))
                        st += 1
                    msb = rscp.tile([128, 128], bf16, tag="msb")
                    nc.scalar.copy(msb[:], mps[:])
                    win_ps = pswinp.tile([128, C_CTX + 1], f32, tag="win")
                    nc.tensor.matmul(win_ps[:], msb[:], fe[:],
                                     start=True, stop=True)
                    tsb = statsp.tile([128, 1], f32, tag="tsb")
                    nc.scalar.copy(tsb[:], win_ps[:, C_CTX:C_CTX + 1])
                    gt = ysbp.tile([128, C_CTX], f32, tag="gt")
                    nc.scalar.activation(gt[:], gam_sb[:], CP, scale=tsb[:])
                    ysb = ysbp.tile([128, C_CTX], f32, tag="ysb")
                    nc.vector.tensor_sub(ysb[:], win_ps[:, 0:C_CTX], gt[:])
                    nc.vector.tensor_add(ysb[:], ysb[:],
                                         ub_sb[:, w * C_CTX:(w + 1) * C_CTX])
                    nc.sync.dma_start(out_d[w * WIN:(w + 1) * WIN, :], ysb[:])
                ti += nts[si]
    nc.compile()
    return nc


TRACE = False          # set by test.py to capture an NTFF profile
TRACE_KW = {}
LAST_RESULT = None     # BassKernelResults stash for test.py


def kernel(**inputs):
    from concourse.bass_utils import run_bass_kernel_spmd

    last_tokens = np.ascontiguousarray(inputs["last_tokens"], dtype=np.float32)
    meta = _host_metadata(inputs["lidar2img"], inputs["w_view"], inputs["logits"],
                          inputs["ln_gamma"], inputs["ln_beta"])
    core_win_pts, core_win_rows, ntw, packs = _build_schedule(meta)
    nc = _build_bass(ntw, packs)

    gamrep = np.broadcast_to(meta['Gam'][None, :], (128, C_CTX)).copy()
    gwrep = np.broadcast_to(meta['gw'][None, :], (128, C)).copy()
    ident = np.eye(128, dtype=np.float32)
    ltb = last_tokens.reshape(NROW, C).astype(_BF16)

    in_maps = []
    for c in range(N_CORES):
        arr = _build_core_arrays(c, meta, core_win_pts, core_win_rows, ntw, packs)
        in_maps.append({
            "ltb": ltb,
            "fidx": arr['fidx'],
            "w1": arr['w1'],
            "w1t": arr['w1t'],
            "r0": arr['r0'],
            "ub": arr['ub'],
            "gamrep": gamrep,
            "gwrep": gwrep,
            "ident": ident,
        })

    res = run_bass_kernel_spmd(nc, in_maps, core_ids=list(range(N_CORES)),
                               trace=TRACE, **TRACE_KW)
    global LAST_RESULT
    LAST_RESULT = res
    out = np.zeros((Q, C_CTX), np.float32)
    for c in range(N_CORES):
        y = res.results[c]["out"]          # [896, 256], q-major
        out[c * QC:(c + 1) * QC] = y[:QC]
    return out.T.reshape(1, C_CTX, BEV_H, BEV_W).copy()


if __name__ == "__main__":
    ins = dict(np.load('/root/problem/inputs_cache.npz'))
    expected = np.load('/root/problem/expected_cache.npy')
    got = kernel(**ins)
    err = np.linalg.norm(got - expected) / np.linalg.norm(expected)
    print("rel_err:", err)
